# revision 47
# baseline (speedup 1.0000x reference)
"""Bahdanau (MLP) attention kernel for Trainium2, data-parallel over 8 NeuronCores.

Math per batch element b (one core each):
    qh[h,q] = sum_d Wq[h,d] query[q,d] + bq[h]          (PE)
    mh[h,m] = sum_d Wm[h,d] memory[m,d]                 (PE)
    t_q[h,m] = tanh(mh[h,m] + qh[h,q])                  (DVE/GPSIMD pre-add + ACT tanh)
    attn[q,m] = sum_h v[h] t_q[h,m]                     (PE, t chunk stationary)
    weights = softmax_m(attn + penalty)                 (DVE reduce + ACT exp)
    wm[q,d] = sum_m weights[q,m] memory[m,d]            (PE, weights transposed via PE)

Key optimizations (322.7us -> ~145us over the tuning session):
  - H=128 in partitions so the q-dependent shift is a per-partition ACT bias /
    DVE scalar; the big (Q,M,H) intermediate never exists in HBM.
  - Masked memory positions produce exactly-0 softmax weights, so unmasked
    columns are gathered host-side (compaction) and results scattered back;
    the device processes MC = ceil(max_unmasked/4)*4 columns (tanh/contract)
    padded to MCP = ceil(MC/128)*128 for the transpose/softmax domain.
  - The +qh pre-add runs on the otherwise-idle DVE in fp16 (2 elem/cycle),
    batching GK q's into one big ACT tanh to amortize per-op overhead
    (ACT = 1 elem/cycle/lane regardless of dtype and is the bottleneck).
  - The mask penalty is a per-partition scalar in the attnT[m, q] layout, so
    it rides the PSUM->SBUF copy for free; softmax skips max-subtraction
    (|attn| <= ~11, exp cannot overflow; -inf pads give exact zeros).
  - fp16 everywhere the PE streams (1 cycle/row vs 4 for fp32): inputs,
    t/v contraction, attn transposes, weights transposes, stage-4 matmul.
    PSUM accumulation stays fp32; overall rel err ~4e-4.
  - First q-group uses ACT bias directly (no pre-add dependency) so tanh
    starts as soon as mh lands; group sizes taper at the end so the PE
    drains with the ACT; per-(m-pair, q-block) PSUM split lets the q<128
    epilogue overlap the q>=128 hot loop; u/t triple-buffered so the DVE
    pre-adds run ahead through the mid-loop epilogue.
  - Epilogue critical path: exp runs directly in the attnT[m, q] layout;
    the softmax sum over m is a PARTITION reduction done on the PE
    (lhsT=eT, rhs=ones -> per-partition [q,1]), and eT is already the lhsT
    layout the stage-4 matmul needs -- no transpose on the wm path at all.
    The attn->[q,m] transposes only feed the weights store, with the 1/sum
    normalization riding the PSUM->SBUF copy as a per-partition scalar.
    DMA issue costs ~650ns each on the sync engine, so critical loads are
    few and large with everything else on the parallel SWDGE queue.
"""

import sys

import numpy as np

sys.path.insert(0, "/opt/trn_rl_repo")

B, QLEN, MLEN = 8, 256, 1024
QS, MS, HID = 256, 256, 128
NCORES = 8
P = 128
NEG = -1.0e30
GK = 16  # q's per tanh batch

_compiled = {}


def _build_bass(MC):
    import concourse.bass as bass
    import concourse.tile as tile
    from concourse import bacc, mybir

    f32 = mybir.dt.float32
    f16 = mybir.dt.float16
    AF = mybir.ActivationFunctionType
    AX = mybir.AxisListType

    n_mc = (MC + P - 1) // P  # chunks incl. partial
    MCP = n_mc * P
    # pack m-chunks per PSUM tile; 3 chunks (1.5KB) still fit one 2KB bank,
    # keeping total PSUM <= 8 banks even for n_mc == 8
    CP = 2 if n_mc <= 6 else 3
    n_pair = (n_mc + CP - 1) // CP
    lastw = MC - (n_mc - 1) * P  # width of last chunk (1..128)

    nc = bacc.Bacc("TRN2", target_bir_lowering=False, debug=False, num_devices=NCORES)

    queryT = nc.dram_tensor("queryT", [QS, QLEN], f16, kind="ExternalInput").ap()
    memT = nc.dram_tensor("memT", [MS, MC], f16, kind="ExternalInput").ap()
    mem = nc.dram_tensor("mem", [MCP, MS], f16, kind="ExternalInput").ap()
    WqT = nc.dram_tensor("WqT", [QS, HID], f16, kind="ExternalInput").ap()
    WmT = nc.dram_tensor("WmT", [MS, HID], f16, kind="ExternalInput").ap()
    bq = nc.dram_tensor("bq", [HID, 1], f32, kind="ExternalInput").ap()
    v = nc.dram_tensor("v", [HID, 1], f16, kind="ExternalInput").ap()
    pen = nc.dram_tensor("pen", [MCP, 1], f32, kind="ExternalInput").ap()
    identd = nc.dram_tensor("identd", [P, P], f16, kind="ExternalInput").ap()
    weights = nc.dram_tensor("weights", [QLEN, MCP], f16, kind="ExternalOutput").ap()
    wm = nc.dram_tensor("wm", [QLEN, MS], f16, kind="ExternalOutput").ap()

    with tile.TileContext(nc) as tc:
        with (
            tc.tile_pool(name="singles", bufs=1) as singles,
            tc.tile_pool(name="upool", bufs=3) as upool,
            tc.tile_pool(name="tpool", bufs=3) as tpool,
            tc.tile_pool(name="soft", bufs=2) as soft,
            tc.tile_pool(name="pattn", bufs=1, space="PSUM") as pattn,
            tc.tile_pool(name="pmm", bufs=2, space="PSUM") as pmm,
        ):
            # ---- load inputs ------------------------------------------------
            # DMA *issue* on the sync engine costs ~650ns each, so the
            # critical path gets few, large dma_starts; everything else rides
            # the (parallel) SWDGE queue via gpsimd.
            memT_sb = singles.tile([P, 2, MC], f16)
            nc.sync.dma_start(
                out=memT_sb, in_=memT.rearrange("(c p) m -> p c m", p=P)
            )
            WmT_sb = singles.tile([P, 2, HID], f16)
            nc.sync.dma_start(out=WmT_sb, in_=WmT.rearrange("(c p) h -> p c h", p=P))
            qT_sb = singles.tile([P, 2, QLEN], f16)
            nc.sync.dma_start(out=qT_sb, in_=queryT.rearrange("(c p) q -> p c q", p=P))
            WqT_sb = singles.tile([P, 2, HID], f16)
            nc.sync.dma_start(out=WqT_sb, in_=WqT.rearrange("(c p) h -> p c h", p=P))
            bq_sb = singles.tile([P, 1], f32)
            nc.gpsimd.dma_start(out=bq_sb, in_=bq)
            v_sb = singles.tile([P, 1], f16)
            nc.gpsimd.dma_start(out=v_sb, in_=v)
            ident16 = singles.tile([P, P], f16)
            nc.gpsimd.dma_start(out=ident16, in_=identd)
            penT_sb = singles.tile([P, n_mc], f32)
            nc.gpsimd.dma_start(
                out=penT_sb, in_=pen.rearrange("(c p) x -> p c x", p=P)
            )
            mem_sb = singles.tile([P, n_mc, MS], f16)
            nc.gpsimd.dma_start(
                out=mem_sb, in_=mem.rearrange("(c p) d -> p c d", p=P)
            )
            ones_sb = singles.tile([P, 1], f16)
            nc.vector.memset(ones_sb, 1.0)

            # ---- PE warm-up while DMAs are in flight ------------------------
            warm_sb = singles.tile([P, P], f16)
            nc.vector.memset(warm_sb, 0.0)
            warm_ps = pmm.tile([P, 1], f32, tag="mm", name="warm_ps")
            for _ in range(30):
                nc.tensor.matmul(
                    out=warm_ps, lhsT=warm_sb, rhs=warm_sb[:, 0:1],
                    start=True, stop=True,
                )

            # ---- mh, qh (mh first: it is the longer pole) -------------------
            mh_sb = singles.tile([P, MC], f16)
            for off in range(0, MC, 512):
                w = min(512, MC - off)
                sl = slice(off, off + w)
                mh_ps = pmm.tile([P, 512], f32, tag="mm", name="mh_ps")
                for c in range(2):
                    nc.tensor.matmul(
                        out=mh_ps[:, :w],
                        lhsT=WmT_sb[:, c, :],
                        rhs=memT_sb[:, c, sl],
                        start=(c == 0),
                        stop=(c == 1),
                    )
                nc.vector.tensor_copy(out=mh_sb[:, sl], in_=mh_ps[:, :w])

            qh_ps = pmm.tile([P, QLEN], f32, tag="mm")
            for c in range(2):
                nc.tensor.matmul(
                    out=qh_ps,
                    lhsT=WqT_sb[:, c, :],
                    rhs=qT_sb[:, c, :],
                    start=(c == 0),
                    stop=(c == 1),
                )
            qh_sb = singles.tile([P, QLEN], f32)
            nc.vector.tensor_scalar_add(qh_sb, qh_ps, bq_sb)

            # ---- hot loop ---------------------------------------------------
            # attnT[m, q] in PSUM, split by (m-chunk pair, q-block) so the
            # q<128 epilogue can run while q>=128 is still being produced.
            attn_ps = {}
            for j in range(n_pair):
                for qb in range(2):
                    attn_ps[(j, qb)] = pattn.tile(
                        [P, CP, P], f32, tag=f"attnT{j}_{qb}", name=f"attnT{j}_{qb}"
                    )

            def epilogue(qb, act_help):
                # attnT psum -> sbuf (fp16) with penalty added as a
                # per-partition scalar; pad partitions get -inf. After the
                # hot loop the ACT engine is idle, so the tail epilogue
                # splits these adds between DVE and ACT (Identity + bias).
                attnT_sb = soft.tile(
                    [P, n_mc, P], f16, tag=f"attnT_sb{qb}", name="attnT_sb"
                )
                if lastw < P:
                    nc.vector.memset(attnT_sb[:, n_mc - 1, :], NEG)
                for mc in range(n_mc):
                    w = P if mc < n_mc - 1 else lastw
                    if act_help and mc % 2 == 1:
                        nc.scalar.activation(
                            out=attnT_sb[:w, mc, :],
                            in_=attn_ps[(mc // CP, qb)][:w, mc % CP, :],
                            func=AF.Identity,
                            bias=penT_sb[:w, mc : mc + 1],
                            scale=1.0,
                        )
                    else:
                        nc.vector.tensor_scalar_add(
                            attnT_sb[:w, mc, :],
                            attn_ps[(mc // CP, qb)][:w, mc % CP, :],
                            penT_sb[:w, mc : mc + 1],
                        )
                # exp directly in the attnT[m, q] layout: pad rows are -inf
                # -> exactly 0. The softmax sum over m is a PARTITION
                # reduction, which the PE does via lhsT=eT, rhs=ones -> the
                # sum lands per-partition [q, 1]; and eT is already the lhsT
                # layout stage-4 wants, so no transpose on the wm path.
                eT_sb = soft.tile([P, n_mc, P], f16, tag=f"eT{qb}", name="eT_sb")
                nc.scalar.activation(out=eT_sb, in_=attnT_sb, func=AF.Exp)
                ssum_ps = pmm.tile([P, 1], f32, tag="mm", name="ssum_ps")
                for mc in range(n_mc):
                    nc.tensor.matmul(
                        out=ssum_ps,
                        lhsT=eT_sb[:, mc, :],
                        rhs=ones_sb,
                        start=(mc == 0),
                        stop=(mc == n_mc - 1),
                    )
                rs = soft.tile([P, 1], f32, tag="rs", name="rs")
                nc.vector.reciprocal(out=rs, in_=ssum_ps)
                # wm = (e @ memory) * r  (fp16 path: 1 PE cycle/row)
                out_ps = pmm.tile([P, MS], f32, tag="mm", name="out_ps")
                for mc in range(n_mc):
                    nc.tensor.matmul(
                        out=out_ps,
                        lhsT=eT_sb[:, mc, :],
                        rhs=mem_sb[:, mc, :],
                        start=(mc == 0),
                        stop=(mc == n_mc - 1),
                    )
                out_sb = soft.tile([P, MS], f16, tag=f"out{qb}", name="out_sb")
                nc.vector.tensor_scalar_mul(out_sb, out_ps, rs)
                for sl in (slice(0, MS // 2), slice(MS // 2, MS)):
                    nc.sync.dma_start(
                        out=wm[qb * P : (qb + 1) * P, sl], in_=out_sb[:, sl]
                    )
                # weights output (off the wm critical path): transpose eT back
                # to [q, m]; the 1/sum normalization rides the PSUM->SBUF copy
                # as a per-partition scalar
                w16_sb = soft.tile([P, MCP], f16, tag=f"w16{qb}", name="w16_sb")
                for mc in range(n_mc):
                    at_ps = pmm.tile([P, P], f16, tag="mm", name="at_ps")
                    nc.tensor.transpose(
                        out=at_ps, in_=eT_sb[:, mc, :], identity=ident16
                    )
                    nc.vector.tensor_scalar_mul(
                        w16_sb[:, mc * P : (mc + 1) * P], at_ps, rs
                    )
                whalf = MCP // 2
                for sl in (slice(0, whalf), slice(whalf, MCP)):
                    nc.sync.dma_start(
                        out=weights[qb * P : (qb + 1) * P, sl], in_=w16_sb[:, sl]
                    )

            gsizes = [(4, True), (4, False), (8, False)] + [(GK, False)] * (
                (QLEN - 32) // GK
            ) + [(8, False), (4, False), (4, False)]
            assert sum(gs for gs, _ in gsizes) == QLEN
            q0 = 0
            for gs, use_bias in gsizes:
                t_sb = tpool.tile([P, GK, MC], f16, tag="t", name="t_sb")
                if use_bias:
                    # startup: tanh with per-partition bias, no pre-add needed
                    for j in range(gs):
                        q = q0 + j
                        nc.scalar.activation(
                            out=t_sb[:, j, :], in_=mh_sb, func=AF.Tanh,
                            bias=qh_sb[:, q : q + 1], scale=1.0,
                        )
                else:
                    u_sb = upool.tile([P, GK, MC], f16, tag="u", name="u_sb")
                    for j in range(gs):
                        q = q0 + j
                        nc.vector.tensor_scalar_add(
                            u_sb[:, j, :], mh_sb, qh_sb[:, q : q + 1]
                        )
                    nc.scalar.activation(
                        out=t_sb[:, :gs, :], in_=u_sb[:, :gs, :], func=AF.Tanh
                    )
                for j in range(gs):
                    q = q0 + j
                    qb, qi = divmod(q, P)
                    for mc in range(n_mc):
                        w = P if mc < n_mc - 1 else lastw
                        nc.tensor.matmul(
                            out=attn_ps[(mc // CP, qb)][:w, mc % CP, qi : qi + 1],
                            lhsT=t_sb[:, j, mc * P : mc * P + w],
                            rhs=v_sb,
                            start=True,
                            stop=True,
                        )
                q0 += gs
                if q0 == P:
                    epilogue(0, act_help=False)
            epilogue(1, act_help=True)

    nc.compile()
    return nc


def _get_nc(MC):
    if MC not in _compiled:
        _compiled[MC] = _build_bass(MC)
    return _compiled[MC]


def kernel(query, memory, Wq, bq, Wm, v, mask, _trace=False):
    from concourse.bass_utils import run_bass_kernel_spmd

    query = np.asarray(query, dtype=np.float32)
    memory = np.asarray(memory, dtype=np.float32)
    Wq = np.asarray(Wq, dtype=np.float32)
    bq = np.asarray(bq, dtype=np.float32)
    Wm = np.asarray(Wm, dtype=np.float32)
    v = np.asarray(v, dtype=np.float32)
    mask = np.asarray(mask).astype(bool)

    idxs = [np.nonzero(~mask[b])[0] for b in range(NCORES)]
    cnts = [len(ix) for ix in idxs]
    MC = max(32, ((max(cnts) + 1) // 2) * 2)
    MC = min(MC, MLEN)
    n_mc = (MC + P - 1) // P
    MCP = n_mc * P

    nc = _get_nc(MC)

    WqT = np.ascontiguousarray(Wq.T).astype(np.float16)
    WmT = np.ascontiguousarray(Wm.T).astype(np.float16)
    bq_c = np.ascontiguousarray(bq.reshape(HID, 1))
    v_c = np.ascontiguousarray(v.reshape(HID, 1)).astype(np.float16)
    ident = np.eye(P, dtype=np.float16)

    in_maps = []
    for b in range(NCORES):
        ix, cnt = idxs[b], cnts[b]
        mem_c = np.zeros((MCP, MS), dtype=np.float32)
        mem_c[:cnt] = memory[b][ix]
        pen_c = np.full((MCP, 1), NEG, dtype=np.float32)
        pen_c[:cnt] = 0.0
        in_maps.append(
            {
                "queryT": np.ascontiguousarray(query[b].T).astype(np.float16),
                "memT": np.ascontiguousarray(mem_c[:MC].T).astype(np.float16),
                "mem": mem_c.astype(np.float16),
                "WqT": WqT,
                "WmT": WmT,
                "bq": bq_c,
                "v": v_c,
                "pen": pen_c,
                "identd": ident,
            }
        )

    res = run_bass_kernel_spmd(nc, in_maps, list(range(NCORES)), trace=_trace)
    results = res.results

    wm_full = np.stack([results[i]["wm"].astype(np.float32) for i in range(NCORES)])
    weights_full = np.zeros((NCORES, QLEN, MLEN), dtype=np.float32)
    for b in range(NCORES):
        weights_full[b][:, idxs[b]] = results[b]["weights"][:, : cnts[b]].astype(np.float32)
    if _trace:
        kernel.last_exec_time_ns = res.exec_time_ns
        kernel.last_trace = res.instructions_and_trace
    return wm_full, weights_full


# revision 48
# speedup vs baseline: 1.0110x; 1.0110x over previous
"""Bahdanau (MLP) attention kernel for Trainium2, data-parallel over 8 NeuronCores.

Math per batch element b (one core each):
    qh[h,q] = sum_d Wq[h,d] query[q,d] + bq[h]          (PE)
    mh[h,m] = sum_d Wm[h,d] memory[m,d]                 (PE)
    t_q[h,m] = tanh(mh[h,m] + qh[h,q])                  (DVE/GPSIMD pre-add + ACT tanh)
    attn[q,m] = sum_h v[h] t_q[h,m]                     (PE, t chunk stationary)
    weights = softmax_m(attn + penalty)                 (DVE reduce + ACT exp)
    wm[q,d] = sum_m weights[q,m] memory[m,d]            (PE, weights transposed via PE)

Key optimizations (322.7us -> ~145us over the tuning session):
  - H=128 in partitions so the q-dependent shift is a per-partition ACT bias /
    DVE scalar; the big (Q,M,H) intermediate never exists in HBM.
  - Masked memory positions produce exactly-0 softmax weights, so unmasked
    columns are gathered host-side (compaction) and results scattered back;
    the device processes MC = ceil(max_unmasked/4)*4 columns (tanh/contract)
    padded to MCP = ceil(MC/128)*128 for the transpose/softmax domain.
  - The +qh pre-add runs on the otherwise-idle DVE in fp16 (2 elem/cycle),
    batching GK q's into one big ACT tanh to amortize per-op overhead
    (ACT = 1 elem/cycle/lane regardless of dtype and is the bottleneck).
  - The mask penalty is a per-partition scalar in the attnT[m, q] layout, so
    it rides the PSUM->SBUF copy for free; softmax skips max-subtraction
    (|attn| <= ~11, exp cannot overflow; -inf pads give exact zeros).
  - fp16 everywhere the PE streams (1 cycle/row vs 4 for fp32): inputs,
    t/v contraction, attn transposes, weights transposes, stage-4 matmul.
    PSUM accumulation stays fp32; overall rel err ~4e-4.
  - First q-group uses ACT bias directly (no pre-add dependency) so tanh
    starts as soon as mh lands; group sizes taper at the end so the PE
    drains with the ACT; per-(m-pair, q-block) PSUM split lets the q<128
    epilogue overlap the q>=128 hot loop; u/t triple-buffered so the DVE
    pre-adds run ahead through the mid-loop epilogue.
  - Epilogue critical path: exp runs directly in the attnT[m, q] layout;
    the softmax sum over m is a PARTITION reduction done on the PE
    (lhsT=eT, rhs=ones -> per-partition [q,1]), and eT is already the lhsT
    layout the stage-4 matmul needs -- no transpose on the wm path at all.
    The attn->[q,m] transposes only feed the weights store, with the 1/sum
    normalization riding the PSUM->SBUF copy as a per-partition scalar.
    DMA issue costs ~650ns each on the sync engine, so critical loads are
    few and large with everything else on the parallel SWDGE queue.
"""

import sys

import numpy as np

sys.path.insert(0, "/opt/trn_rl_repo")

B, QLEN, MLEN = 8, 256, 1024
QS, MS, HID = 256, 256, 128
NCORES = 8
P = 128
NEG = -1.0e30
GK = 16  # q's per tanh batch

_compiled = {}


def _build_bass(MC):
    import concourse.bass as bass
    import concourse.tile as tile
    from concourse import bacc, mybir

    f32 = mybir.dt.float32
    f16 = mybir.dt.float16
    AF = mybir.ActivationFunctionType
    AX = mybir.AxisListType

    n_mc = (MC + P - 1) // P  # chunks incl. partial
    MCP = n_mc * P
    # pack m-chunks per PSUM tile; 3 chunks (1.5KB) still fit one 2KB bank,
    # keeping total PSUM <= 8 banks even for n_mc == 8
    CP = 2 if n_mc <= 6 else 3
    n_pair = (n_mc + CP - 1) // CP
    lastw = MC - (n_mc - 1) * P  # width of last chunk (1..128)

    nc = bacc.Bacc("TRN2", target_bir_lowering=False, debug=False, num_devices=NCORES)

    queryT = nc.dram_tensor("queryT", [QS, QLEN], f16, kind="ExternalInput").ap()
    memT = nc.dram_tensor("memT", [MS, MC], f16, kind="ExternalInput").ap()
    mem = nc.dram_tensor("mem", [MCP, MS], f16, kind="ExternalInput").ap()
    WqT = nc.dram_tensor("WqT", [QS, HID], f16, kind="ExternalInput").ap()
    WmT = nc.dram_tensor("WmT", [MS, HID], f16, kind="ExternalInput").ap()
    bq = nc.dram_tensor("bq", [HID, 1], f32, kind="ExternalInput").ap()
    v = nc.dram_tensor("v", [HID, 1], f16, kind="ExternalInput").ap()
    pen = nc.dram_tensor("pen", [MCP, 1], f32, kind="ExternalInput").ap()
    identd = nc.dram_tensor("identd", [P, P], f16, kind="ExternalInput").ap()
    weights = nc.dram_tensor("weights", [QLEN, MCP], f16, kind="ExternalOutput").ap()
    wm = nc.dram_tensor("wm", [QLEN, MS], f16, kind="ExternalOutput").ap()

    with tile.TileContext(nc) as tc:
        with (
            tc.tile_pool(name="singles", bufs=1) as singles,
            tc.tile_pool(name="upool", bufs=3) as upool,
            tc.tile_pool(name="tpool", bufs=3) as tpool,
            tc.tile_pool(name="soft", bufs=2) as soft,
            tc.tile_pool(name="pattn", bufs=1, space="PSUM") as pattn,
            tc.tile_pool(name="pmm", bufs=2, space="PSUM") as pmm,
        ):
            # ---- load inputs ------------------------------------------------
            # DMA *issue* on the sync engine costs ~650ns each, so the
            # critical path gets few, large dma_starts; everything else rides
            # the (parallel) SWDGE queue via gpsimd.
            memT_sb = singles.tile([P, 2, MC], f16)
            nc.sync.dma_start(
                out=memT_sb, in_=memT.rearrange("(c p) m -> p c m", p=P)
            )
            WmT_sb = singles.tile([P, 2, HID], f16)
            nc.sync.dma_start(out=WmT_sb, in_=WmT.rearrange("(c p) h -> p c h", p=P))
            qT_sb = singles.tile([P, 2, QLEN], f16)
            nc.sync.dma_start(out=qT_sb, in_=queryT.rearrange("(c p) q -> p c q", p=P))
            WqT_sb = singles.tile([P, 2, HID], f16)
            nc.sync.dma_start(out=WqT_sb, in_=WqT.rearrange("(c p) h -> p c h", p=P))
            bq_sb = singles.tile([P, 1], f32)
            nc.gpsimd.dma_start(out=bq_sb, in_=bq)
            v_sb = singles.tile([P, 1], f16)
            nc.gpsimd.dma_start(out=v_sb, in_=v)
            ident16 = singles.tile([P, P], f16)
            nc.gpsimd.dma_start(out=ident16, in_=identd)
            penT_sb = singles.tile([P, n_mc], f32)
            nc.gpsimd.dma_start(
                out=penT_sb, in_=pen.rearrange("(c p) x -> p c x", p=P)
            )
            mem_sb = singles.tile([P, n_mc, MS], f16)
            nc.gpsimd.dma_start(
                out=mem_sb, in_=mem.rearrange("(c p) d -> p c d", p=P)
            )
            ones_sb = singles.tile([P, 1], f16)
            nc.vector.memset(ones_sb, 1.0)

            # ---- mh, qh (mh first: it is the longer pole) -------------------
            mh_sb = singles.tile([P, MC], f16)
            for off in range(0, MC, 512):
                w = min(512, MC - off)
                sl = slice(off, off + w)
                mh_ps = pmm.tile([P, 512], f32, tag="mm", name="mh_ps")
                for c in range(2):
                    nc.tensor.matmul(
                        out=mh_ps[:, :w],
                        lhsT=WmT_sb[:, c, :],
                        rhs=memT_sb[:, c, sl],
                        start=(c == 0),
                        stop=(c == 1),
                    )
                nc.vector.tensor_copy(out=mh_sb[:, sl], in_=mh_ps[:, :w])

            qh_ps = pmm.tile([P, QLEN], f32, tag="mm")
            for c in range(2):
                nc.tensor.matmul(
                    out=qh_ps,
                    lhsT=WqT_sb[:, c, :],
                    rhs=qT_sb[:, c, :],
                    start=(c == 0),
                    stop=(c == 1),
                )
            qh_sb = singles.tile([P, QLEN], f32)
            nc.vector.tensor_scalar_add(qh_sb, qh_ps, bq_sb)

            # ---- hot loop ---------------------------------------------------
            # attnT[m, q] in PSUM, split by (m-chunk pair, q-block) so the
            # q<128 epilogue can run while q>=128 is still being produced.
            attn_ps = {}
            for j in range(n_pair):
                for qb in range(2):
                    attn_ps[(j, qb)] = pattn.tile(
                        [P, CP, P], f32, tag=f"attnT{j}_{qb}", name=f"attnT{j}_{qb}"
                    )

            def epilogue(qb, act_help):
                # attnT psum -> sbuf (fp16) with penalty added as a
                # per-partition scalar; pad partitions get -inf. After the
                # hot loop the ACT engine is idle, so the tail epilogue
                # splits these adds between DVE and ACT (Identity + bias).
                attnT_sb = soft.tile(
                    [P, n_mc, P], f16, tag=f"attnT_sb{qb}", name="attnT_sb"
                )
                if lastw < P:
                    nc.vector.memset(attnT_sb[:, n_mc - 1, :], NEG)
                for mc in range(n_mc):
                    w = P if mc < n_mc - 1 else lastw
                    if act_help and mc % 2 == 1:
                        nc.scalar.activation(
                            out=attnT_sb[:w, mc, :],
                            in_=attn_ps[(mc // CP, qb)][:w, mc % CP, :],
                            func=AF.Identity,
                            bias=penT_sb[:w, mc : mc + 1],
                            scale=1.0,
                        )
                    else:
                        nc.vector.tensor_scalar_add(
                            attnT_sb[:w, mc, :],
                            attn_ps[(mc // CP, qb)][:w, mc % CP, :],
                            penT_sb[:w, mc : mc + 1],
                        )
                # exp directly in the attnT[m, q] layout: pad rows are -inf
                # -> exactly 0. The softmax sum over m is a PARTITION
                # reduction, which the PE does via lhsT=eT, rhs=ones -> the
                # sum lands per-partition [q, 1]; and eT is already the lhsT
                # layout stage-4 wants, so no transpose on the wm path.
                eT_sb = soft.tile([P, n_mc, P], f16, tag=f"eT{qb}", name="eT_sb")
                nc.scalar.activation(out=eT_sb, in_=attnT_sb, func=AF.Exp)
                ssum_ps = pmm.tile([P, 1], f32, tag="mm", name="ssum_ps")
                for mc in range(n_mc):
                    nc.tensor.matmul(
                        out=ssum_ps,
                        lhsT=eT_sb[:, mc, :],
                        rhs=ones_sb,
                        start=(mc == 0),
                        stop=(mc == n_mc - 1),
                    )
                rs = soft.tile([P, 1], f32, tag="rs", name="rs")
                nc.vector.reciprocal(out=rs, in_=ssum_ps)
                # wm = (e @ memory) * r  (fp16 path: 1 PE cycle/row)
                out_ps = pmm.tile([P, MS], f32, tag="mm", name="out_ps")
                for mc in range(n_mc):
                    nc.tensor.matmul(
                        out=out_ps,
                        lhsT=eT_sb[:, mc, :],
                        rhs=mem_sb[:, mc, :],
                        start=(mc == 0),
                        stop=(mc == n_mc - 1),
                    )
                out_sb = soft.tile([P, MS], f16, tag=f"out{qb}", name="out_sb")
                nc.vector.tensor_scalar_mul(out_sb, out_ps, rs)
                for sl in (slice(0, MS // 2), slice(MS // 2, MS)):
                    nc.sync.dma_start(
                        out=wm[qb * P : (qb + 1) * P, sl], in_=out_sb[:, sl]
                    )
                # weights output (off the wm critical path): transpose eT back
                # to [q, m]; the 1/sum normalization rides the PSUM->SBUF copy
                # as a per-partition scalar
                w16_sb = soft.tile([P, MCP], f16, tag=f"w16{qb}", name="w16_sb")
                for mc in range(n_mc):
                    at_ps = pmm.tile([P, P], f16, tag="mm", name="at_ps")
                    nc.tensor.transpose(
                        out=at_ps, in_=eT_sb[:, mc, :], identity=ident16
                    )
                    nc.vector.tensor_scalar_mul(
                        w16_sb[:, mc * P : (mc + 1) * P], at_ps, rs
                    )
                whalf = MCP // 2
                for sl in (slice(0, whalf), slice(whalf, MCP)):
                    nc.sync.dma_start(
                        out=weights[qb * P : (qb + 1) * P, sl], in_=w16_sb[:, sl]
                    )

            gsizes = [(4, True), (4, False), (8, False)] + [(GK, False)] * (
                (QLEN - 32) // GK
            ) + [(8, False), (4, False), (4, False)]
            assert sum(gs for gs, _ in gsizes) == QLEN
            q0 = 0
            for gs, use_bias in gsizes:
                t_sb = tpool.tile([P, GK, MC], f16, tag="t", name="t_sb")
                if use_bias:
                    # startup: tanh with per-partition bias, no pre-add needed
                    for j in range(gs):
                        q = q0 + j
                        nc.scalar.activation(
                            out=t_sb[:, j, :], in_=mh_sb, func=AF.Tanh,
                            bias=qh_sb[:, q : q + 1], scale=1.0,
                        )
                else:
                    u_sb = upool.tile([P, GK, MC], f16, tag="u", name="u_sb")
                    for j in range(gs):
                        q = q0 + j
                        nc.vector.tensor_scalar_add(
                            u_sb[:, j, :], mh_sb, qh_sb[:, q : q + 1]
                        )
                    nc.scalar.activation(
                        out=t_sb[:, :gs, :], in_=u_sb[:, :gs, :], func=AF.Tanh
                    )
                for j in range(gs):
                    q = q0 + j
                    qb, qi = divmod(q, P)
                    for mc in range(n_mc):
                        w = P if mc < n_mc - 1 else lastw
                        nc.tensor.matmul(
                            out=attn_ps[(mc // CP, qb)][:w, mc % CP, qi : qi + 1],
                            lhsT=t_sb[:, j, mc * P : mc * P + w],
                            rhs=v_sb,
                            start=True,
                            stop=True,
                        )
                q0 += gs
                if q0 == P:
                    epilogue(0, act_help=False)
            epilogue(1, act_help=True)

    nc.compile()
    return nc


def _get_nc(MC):
    if MC not in _compiled:
        _compiled[MC] = _build_bass(MC)
    return _compiled[MC]


def kernel(query, memory, Wq, bq, Wm, v, mask, _trace=False):
    from concourse.bass_utils import run_bass_kernel_spmd

    query = np.asarray(query, dtype=np.float32)
    memory = np.asarray(memory, dtype=np.float32)
    Wq = np.asarray(Wq, dtype=np.float32)
    bq = np.asarray(bq, dtype=np.float32)
    Wm = np.asarray(Wm, dtype=np.float32)
    v = np.asarray(v, dtype=np.float32)
    mask = np.asarray(mask).astype(bool)

    idxs = [np.nonzero(~mask[b])[0] for b in range(NCORES)]
    cnts = [len(ix) for ix in idxs]
    MC = max(32, ((max(cnts) + 1) // 2) * 2)
    MC = min(MC, MLEN)
    n_mc = (MC + P - 1) // P
    MCP = n_mc * P

    nc = _get_nc(MC)

    WqT = np.ascontiguousarray(Wq.T).astype(np.float16)
    WmT = np.ascontiguousarray(Wm.T).astype(np.float16)
    bq_c = np.ascontiguousarray(bq.reshape(HID, 1))
    v_c = np.ascontiguousarray(v.reshape(HID, 1)).astype(np.float16)
    ident = np.eye(P, dtype=np.float16)

    in_maps = []
    for b in range(NCORES):
        ix, cnt = idxs[b], cnts[b]
        mem_c = np.zeros((MCP, MS), dtype=np.float32)
        mem_c[:cnt] = memory[b][ix]
        pen_c = np.full((MCP, 1), NEG, dtype=np.float32)
        pen_c[:cnt] = 0.0
        in_maps.append(
            {
                "queryT": np.ascontiguousarray(query[b].T).astype(np.float16),
                "memT": np.ascontiguousarray(mem_c[:MC].T).astype(np.float16),
                "mem": mem_c.astype(np.float16),
                "WqT": WqT,
                "WmT": WmT,
                "bq": bq_c,
                "v": v_c,
                "pen": pen_c,
                "identd": ident,
            }
        )

    res = run_bass_kernel_spmd(nc, in_maps, list(range(NCORES)), trace=_trace)
    results = res.results

    wm_full = np.stack([results[i]["wm"].astype(np.float32) for i in range(NCORES)])
    weights_full = np.zeros((NCORES, QLEN, MLEN), dtype=np.float32)
    for b in range(NCORES):
        weights_full[b][:, idxs[b]] = results[b]["weights"][:, : cnts[b]].astype(np.float32)
    if _trace:
        kernel.last_exec_time_ns = res.exec_time_ns
        kernel.last_trace = res.instructions_and_trace
    return wm_full, weights_full


# revision 49
# speedup vs baseline: 1.0129x; 1.0019x over previous
"""Bahdanau (MLP) attention kernel for Trainium2, data-parallel over 8 NeuronCores.

Math per batch element b (one core each):
    qh[h,q] = sum_d Wq[h,d] query[q,d] + bq[h]          (PE)
    mh[h,m] = sum_d Wm[h,d] memory[m,d]                 (PE)
    t_q[h,m] = tanh(mh[h,m] + qh[h,q])                  (DVE/GPSIMD pre-add + ACT tanh)
    attn[q,m] = sum_h v[h] t_q[h,m]                     (PE, t chunk stationary)
    weights = softmax_m(attn + penalty)                 (DVE reduce + ACT exp)
    wm[q,d] = sum_m weights[q,m] memory[m,d]            (PE, weights transposed via PE)

Key optimizations (322.7us -> ~145us over the tuning session):
  - H=128 in partitions so the q-dependent shift is a per-partition ACT bias /
    DVE scalar; the big (Q,M,H) intermediate never exists in HBM.
  - Masked memory positions produce exactly-0 softmax weights, so unmasked
    columns are gathered host-side (compaction) and results scattered back;
    the device processes MC = ceil(max_unmasked/2)*2 columns (tanh/contract)
    padded to MCP = ceil(MC/128)*128 for the transpose/softmax domain.
  - The +qh pre-add runs on the otherwise-idle DVE in fp16 (2 elem/cycle),
    batching GK q's into one big ACT tanh to amortize per-op overhead
    (ACT = 1 elem/cycle/lane regardless of dtype and is the bottleneck).
  - The mask penalty is a per-partition scalar in the attnT[m, q] layout, so
    it rides the PSUM->SBUF copy for free; softmax skips max-subtraction
    (|attn| <= ~11, exp cannot overflow; -inf pads give exact zeros).
  - fp16 everywhere the PE streams (1 cycle/row vs 4 for fp32): inputs,
    t/v contraction, attn transposes, weights transposes, stage-4 matmul.
    PSUM accumulation stays fp32; overall rel err ~4e-4.
  - First q-group uses ACT bias directly (no pre-add dependency) so tanh
    starts as soon as mh lands; group sizes taper at the end so the PE
    drains with the ACT; per-(m-pair, q-block) PSUM split lets the q<128
    epilogue overlap the q>=128 hot loop; u/t triple-buffered so the DVE
    pre-adds run ahead through the mid-loop epilogue.
  - Epilogue critical path: exp runs directly in the attnT[m, q] layout;
    the softmax sum over m is a PARTITION reduction done on the PE
    (lhsT=eT, rhs=ones -> per-partition [q,1]), and eT is already the lhsT
    layout the stage-4 matmul needs -- no transpose on the wm path at all.
    The attn->[q,m] transposes only feed the weights store, with the 1/sum
    normalization riding the PSUM->SBUF copy as a per-partition scalar.
    DMA issue costs ~650ns each on the sync engine, so critical loads are
    few and large with everything else on the parallel SWDGE queue.
"""

import sys

import numpy as np

sys.path.insert(0, "/opt/trn_rl_repo")

B, QLEN, MLEN = 8, 256, 1024
QS, MS, HID = 256, 256, 128
NCORES = 8
P = 128
NEG = -1.0e30
GK = 16  # q's per tanh batch

_compiled = {}


def _build_bass(MC):
    import concourse.bass as bass
    import concourse.tile as tile
    from concourse import bacc, mybir

    f32 = mybir.dt.float32
    f16 = mybir.dt.float16
    AF = mybir.ActivationFunctionType
    AX = mybir.AxisListType

    n_mc = (MC + P - 1) // P  # chunks incl. partial
    MCP = n_mc * P
    # pack m-chunks per PSUM tile; 3 chunks (1.5KB) still fit one 2KB bank,
    # keeping total PSUM <= 8 banks even for n_mc == 8
    CP = 2 if n_mc <= 6 else 3
    n_pair = (n_mc + CP - 1) // CP
    lastw = MC - (n_mc - 1) * P  # width of last chunk (1..128)

    nc = bacc.Bacc("TRN2", target_bir_lowering=False, debug=False, num_devices=NCORES)

    queryT = nc.dram_tensor("queryT", [QS, QLEN], f16, kind="ExternalInput").ap()
    memT = nc.dram_tensor("memT", [MS, MC], f16, kind="ExternalInput").ap()
    mem = nc.dram_tensor("mem", [MCP, MS], f16, kind="ExternalInput").ap()
    WqT = nc.dram_tensor("WqT", [QS, HID], f16, kind="ExternalInput").ap()
    WmT = nc.dram_tensor("WmT", [MS, HID], f16, kind="ExternalInput").ap()
    bq = nc.dram_tensor("bq", [HID, 1], f32, kind="ExternalInput").ap()
    v = nc.dram_tensor("v", [HID, 1], f16, kind="ExternalInput").ap()
    pen = nc.dram_tensor("pen", [MCP, 1], f32, kind="ExternalInput").ap()
    identd = nc.dram_tensor("identd", [P, P], f16, kind="ExternalInput").ap()
    weights = nc.dram_tensor("weights", [QLEN, MCP], f16, kind="ExternalOutput").ap()
    wm = nc.dram_tensor("wm", [QLEN, MS], f16, kind="ExternalOutput").ap()

    with tile.TileContext(nc) as tc:
        with (
            tc.tile_pool(name="singles", bufs=1) as singles,
            tc.tile_pool(name="upool", bufs=3) as upool,
            tc.tile_pool(name="tpool", bufs=3) as tpool,
            tc.tile_pool(name="soft", bufs=2) as soft,
            tc.tile_pool(name="pattn", bufs=1, space="PSUM") as pattn,
            tc.tile_pool(name="pmm", bufs=2, space="PSUM") as pmm,
        ):
            # ---- load inputs ------------------------------------------------
            # DMA *issue* on the sync engine costs ~650ns each, so the
            # critical path gets few, large dma_starts; everything else rides
            # the (parallel) SWDGE queue via gpsimd.
            memT_sb = singles.tile([P, 2, MC], f16)
            nc.sync.dma_start(
                out=memT_sb, in_=memT.rearrange("(c p) m -> p c m", p=P)
            )
            WmT_sb = singles.tile([P, 2, HID], f16)
            nc.sync.dma_start(out=WmT_sb, in_=WmT.rearrange("(c p) h -> p c h", p=P))
            qT_sb = singles.tile([P, 2, QLEN], f16)
            nc.sync.dma_start(out=qT_sb, in_=queryT.rearrange("(c p) q -> p c q", p=P))
            WqT_sb = singles.tile([P, 2, HID], f16)
            nc.sync.dma_start(out=WqT_sb, in_=WqT.rearrange("(c p) h -> p c h", p=P))
            bq_sb = singles.tile([P, 1], f32)
            nc.gpsimd.dma_start(out=bq_sb, in_=bq)
            v_sb = singles.tile([P, 1], f16)
            nc.gpsimd.dma_start(out=v_sb, in_=v)
            ident16 = singles.tile([P, P], f16)
            nc.gpsimd.dma_start(out=ident16, in_=identd)
            penT_sb = singles.tile([P, n_mc], f32)
            nc.gpsimd.dma_start(
                out=penT_sb, in_=pen.rearrange("(c p) x -> p c x", p=P)
            )
            mem_sb = singles.tile([P, n_mc, MS], f16)
            nc.gpsimd.dma_start(
                out=mem_sb, in_=mem.rearrange("(c p) d -> p c d", p=P)
            )
            ones_sb = singles.tile([P, 1], f16)
            nc.vector.memset(ones_sb, 1.0)

            # ---- mh, qh (mh first: it is the longer pole) -------------------
            mh_sb = singles.tile([P, MC], f16)
            for off in range(0, MC, 512):
                w = min(512, MC - off)
                sl = slice(off, off + w)
                mh_ps = pmm.tile([P, 512], f32, tag="mm", name="mh_ps")
                for c in range(2):
                    nc.tensor.matmul(
                        out=mh_ps[:, :w],
                        lhsT=WmT_sb[:, c, :],
                        rhs=memT_sb[:, c, sl],
                        start=(c == 0),
                        stop=(c == 1),
                    )
                nc.vector.tensor_copy(out=mh_sb[:, sl], in_=mh_ps[:, :w])

            qh_ps = pmm.tile([P, QLEN], f32, tag="mm")
            for c in range(2):
                nc.tensor.matmul(
                    out=qh_ps,
                    lhsT=WqT_sb[:, c, :],
                    rhs=qT_sb[:, c, :],
                    start=(c == 0),
                    stop=(c == 1),
                )
            qh_sb = singles.tile([P, QLEN], f32)
            nc.vector.tensor_scalar_add(qh_sb, qh_ps, bq_sb)

            # ---- hot loop ---------------------------------------------------
            # attnT[m, q] in PSUM, split by (m-chunk pair, q-block) so the
            # q<128 epilogue can run while q>=128 is still being produced.
            attn_ps = {}
            for j in range(n_pair):
                for qb in range(2):
                    attn_ps[(j, qb)] = pattn.tile(
                        [P, CP, P], f32, tag=f"attnT{j}_{qb}", name=f"attnT{j}_{qb}"
                    )

            def epilogue(qb, act_help):
                # attnT psum -> sbuf (fp16) with penalty added as a
                # per-partition scalar; pad partitions get -inf. After the
                # hot loop the ACT engine is idle, so the tail epilogue
                # splits these adds between DVE and ACT (Identity + bias).
                attnT_sb = soft.tile(
                    [P, n_mc, P], f16, tag=f"attnT_sb{qb}", name="attnT_sb"
                )
                if lastw < P:
                    nc.vector.memset(attnT_sb[:, n_mc - 1, :], NEG)
                for mc in range(n_mc):
                    w = P if mc < n_mc - 1 else lastw
                    if act_help and mc % 2 == 1:
                        nc.scalar.activation(
                            out=attnT_sb[:w, mc, :],
                            in_=attn_ps[(mc // CP, qb)][:w, mc % CP, :],
                            func=AF.Identity,
                            bias=penT_sb[:w, mc : mc + 1],
                            scale=1.0,
                        )
                    else:
                        nc.vector.tensor_scalar_add(
                            attnT_sb[:w, mc, :],
                            attn_ps[(mc // CP, qb)][:w, mc % CP, :],
                            penT_sb[:w, mc : mc + 1],
                        )
                # exp directly in the attnT[m, q] layout: pad rows are -inf
                # -> exactly 0. The softmax sum over m is a PARTITION
                # reduction, which the PE does via lhsT=eT, rhs=ones -> the
                # sum lands per-partition [q, 1]; and eT is already the lhsT
                # layout stage-4 wants, so no transpose on the wm path.
                eT_sb = soft.tile([P, n_mc, P], f16, tag=f"eT{qb}", name="eT_sb")
                nc.scalar.activation(out=eT_sb, in_=attnT_sb, func=AF.Exp)
                ssum_ps = pmm.tile([P, 1], f32, tag="mm", name="ssum_ps")
                for mc in range(n_mc):
                    nc.tensor.matmul(
                        out=ssum_ps,
                        lhsT=eT_sb[:, mc, :],
                        rhs=ones_sb,
                        start=(mc == 0),
                        stop=(mc == n_mc - 1),
                    )
                rs = soft.tile([P, 1], f32, tag="rs", name="rs")
                nc.vector.reciprocal(out=rs, in_=ssum_ps)
                # wm = (e @ memory) * r  (fp16 path: 1 PE cycle/row)
                out_ps = pmm.tile([P, MS], f32, tag="mm", name="out_ps")
                for mc in range(n_mc):
                    nc.tensor.matmul(
                        out=out_ps,
                        lhsT=eT_sb[:, mc, :],
                        rhs=mem_sb[:, mc, :],
                        start=(mc == 0),
                        stop=(mc == n_mc - 1),
                    )
                out_sb = soft.tile([P, MS], f16, tag=f"out{qb}", name="out_sb")
                nc.vector.tensor_scalar_mul(out_sb, out_ps, rs)
                for sl in (slice(0, MS // 2), slice(MS // 2, MS)):
                    nc.sync.dma_start(
                        out=wm[qb * P : (qb + 1) * P, sl], in_=out_sb[:, sl]
                    )
                # weights output (off the wm critical path): transpose eT back
                # to [q, m]; the 1/sum normalization rides the PSUM->SBUF copy
                # as a per-partition scalar
                w16_sb = soft.tile([P, MCP], f16, tag=f"w16{qb}", name="w16_sb")
                for mc in range(n_mc):
                    at_ps = pmm.tile([P, P], f16, tag="mm", name="at_ps")
                    nc.tensor.transpose(
                        out=at_ps, in_=eT_sb[:, mc, :], identity=ident16
                    )
                    nc.vector.tensor_scalar_mul(
                        w16_sb[:, mc * P : (mc + 1) * P], at_ps, rs
                    )
                whalf = MCP // 2
                for sl in (slice(0, whalf), slice(whalf, MCP)):
                    nc.sync.dma_start(
                        out=weights[qb * P : (qb + 1) * P, sl], in_=w16_sb[:, sl]
                    )

            gsizes = [(4, True), (4, False), (8, False)] + [(GK, False)] * (
                (QLEN - 32) // GK
            ) + [(8, False), (4, False), (4, False)]
            assert sum(gs for gs, _ in gsizes) == QLEN
            q0 = 0
            for gs, use_bias in gsizes:
                t_sb = tpool.tile([P, GK, MC], f16, tag="t", name="t_sb")
                if use_bias:
                    # startup: tanh with per-partition bias, no pre-add needed
                    for j in range(gs):
                        q = q0 + j
                        nc.scalar.activation(
                            out=t_sb[:, j, :], in_=mh_sb, func=AF.Tanh,
                            bias=qh_sb[:, q : q + 1], scale=1.0,
                        )
                else:
                    u_sb = upool.tile([P, GK, MC], f16, tag="u", name="u_sb")
                    for j in range(gs):
                        q = q0 + j
                        nc.vector.tensor_scalar_add(
                            u_sb[:, j, :], mh_sb, qh_sb[:, q : q + 1]
                        )
                    nc.scalar.activation(
                        out=t_sb[:, :gs, :], in_=u_sb[:, :gs, :], func=AF.Tanh
                    )
                for j in range(gs):
                    q = q0 + j
                    qb, qi = divmod(q, P)
                    for mc in range(n_mc):
                        w = P if mc < n_mc - 1 else lastw
                        nc.tensor.matmul(
                            out=attn_ps[(mc // CP, qb)][:w, mc % CP, qi : qi + 1],
                            lhsT=t_sb[:, j, mc * P : mc * P + w],
                            rhs=v_sb,
                            start=True,
                            stop=True,
                        )
                q0 += gs
                if q0 == P:
                    epilogue(0, act_help=False)
            epilogue(1, act_help=True)

    nc.compile()
    return nc


def _get_nc(MC):
    if MC not in _compiled:
        _compiled[MC] = _build_bass(MC)
    return _compiled[MC]


def kernel(query, memory, Wq, bq, Wm, v, mask, _trace=False):
    from concourse.bass_utils import run_bass_kernel_spmd

    query = np.asarray(query, dtype=np.float32)
    memory = np.asarray(memory, dtype=np.float32)
    Wq = np.asarray(Wq, dtype=np.float32)
    bq = np.asarray(bq, dtype=np.float32)
    Wm = np.asarray(Wm, dtype=np.float32)
    v = np.asarray(v, dtype=np.float32)
    mask = np.asarray(mask).astype(bool)

    idxs = [np.nonzero(~mask[b])[0] for b in range(NCORES)]
    cnts = [len(ix) for ix in idxs]
    MC = max(32, ((max(cnts) + 1) // 2) * 2)
    MC = min(MC, MLEN)
    n_mc = (MC + P - 1) // P
    MCP = n_mc * P

    nc = _get_nc(MC)

    WqT = np.ascontiguousarray(Wq.T).astype(np.float16)
    WmT = np.ascontiguousarray(Wm.T).astype(np.float16)
    bq_c = np.ascontiguousarray(bq.reshape(HID, 1))
    v_c = np.ascontiguousarray(v.reshape(HID, 1)).astype(np.float16)
    ident = np.eye(P, dtype=np.float16)

    in_maps = []
    for b in range(NCORES):
        ix, cnt = idxs[b], cnts[b]
        mem_c = np.zeros((MCP, MS), dtype=np.float32)
        mem_c[:cnt] = memory[b][ix]
        pen_c = np.full((MCP, 1), NEG, dtype=np.float32)
        pen_c[:cnt] = 0.0
        in_maps.append(
            {
                "queryT": np.ascontiguousarray(query[b].T).astype(np.float16),
                "memT": np.ascontiguousarray(mem_c[:MC].T).astype(np.float16),
                "mem": mem_c.astype(np.float16),
                "WqT": WqT,
                "WmT": WmT,
                "bq": bq_c,
                "v": v_c,
                "pen": pen_c,
                "identd": ident,
            }
        )

    res = run_bass_kernel_spmd(nc, in_maps, list(range(NCORES)), trace=_trace)
    results = res.results

    wm_full = np.stack([results[i]["wm"].astype(np.float32) for i in range(NCORES)])
    weights_full = np.zeros((NCORES, QLEN, MLEN), dtype=np.float32)
    for b in range(NCORES):
        weights_full[b][:, idxs[b]] = results[b]["weights"][:, : cnts[b]].astype(np.float32)
    if _trace:
        kernel.last_exec_time_ns = res.exec_time_ns
        kernel.last_trace = res.instructions_and_trace
    return wm_full, weights_full
